# revision 2
# baseline (speedup 1.0000x reference)
"""Trainium2 kernel for nn_LowRank (sparse_attention).

Strategy: data-parallel over batch B=128 across 8 NeuronCores (16 rows each).
The two dominant Linear projections (key/value2: 2 x 137 GMAC, ~95% of FLOPs)
run on-device:
  - k-side in fp8-e4m3 with DoubleRow perf mode (2 MACs/cell/cycle).  A
    numerical simulation shows k-side fp8 is safe (2.2e-3 end-to-end vs
    2.1e-3 for bf16): k-path errors are damped by the near-uniform softmax
    and the sigmoid channel gate.  Weights are pre-scaled by 64 to stay out
    of fp8 subnormals; the PSUM->SBUF copy divides it back out.
  - v2-side in bf16 (fp8 here fails the tolerance: v2 feeds the output
    directly).
The cheap epilogue (CELU, GroupNorm, SCAttention) runs on host in fp32.
"""

import sys

for p in ("/opt/trn_rl_repo",):
    if p not in sys.path:
        sys.path.insert(0, p)

import numpy as np
import ml_dtypes

import concourse.bass as bass
import concourse.bacc as bacc
import concourse.mybir as mybir
from concourse import tile
from concourse.bass_utils import run_bass_kernel_spmd

BF16 = ml_dtypes.bfloat16
F8 = ml_dtypes.float8_e4m3

B, M, E, H, MEM = 128, 1024, 1024, 8, 40
D = E // H
MID = 64
ALPHA = 1.3
EPS = 1e-5
NC = 8
BPC = B // NC          # 16 batch rows per core
R = BPC * M            # 16384 rows of x per core
KC = E // 128          # 8 contraction chunks
W_SCALE = 64.0         # fp8 weight pre-scale (keeps W out of subnormals)
FP8_K = True           # k-side projection in fp8 DoubleRow

# k-side (Layout A: features on partitions, weight-stationary, fp8 DR)
GA = 4                 # psum tiles per stationary weight load
NBLK = 512             # columns (rows of x) per psum tile
NGA = R // (NBLK * GA) # 8 column groups

# v2-side (Layout B: rows on partitions, bf16)
RG = 8                 # row-blocks of 128 per DMA group
NGB = R // (128 * RG)  # 16 groups

_CACHE = {}
TRACE = False          # test.py sets True to capture an NTFF profile
TRACE_DIR = None
VERBOSE = False


def _tick(msg, t0):
    import time
    t = time.time()
    if VERBOSE:
        print(f"[kernel] {msg}: {t - t0:.2f}s", flush=True)
    return t


def _build_nc(fp8_k=FP8_K):
    OP = mybir.AluOpType
    nc = bacc.Bacc(trn_type="TRN2")
    dt_k = mybir.dt.float8e4 if fp8_k else mybir.dt.bfloat16
    xk = nc.dram_tensor("xk", (E, R), dt_k, kind="ExternalInput")
    wk = nc.dram_tensor("wk", (E, E), dt_k, kind="ExternalInput")
    yk = nc.dram_tensor("yk", (E, R), mybir.dt.bfloat16, kind="ExternalOutput")
    xv = nc.dram_tensor("xv", (E, R), mybir.dt.bfloat16, kind="ExternalInput")
    wv = nc.dram_tensor("wv", (E, E), mybir.dt.bfloat16, kind="ExternalInput")
    yv = nc.dram_tensor("yv", (R, E), mybir.dt.bfloat16, kind="ExternalOutput")

    # k-side column groups ramp up so the first matmul only gates on a small
    # DMA payload (wk chunk 0 + a 512-col x tile) instead of 3 MB.
    kwidths = [512, 512, 1024] + [2048] * 7
    assert sum(kwidths) == R

    with tile.TileContext(nc) as tc:
        with (
            tc.tile_pool(name="wpool", bufs=1) as wpool,
            tc.tile_pool(name="xpool", bufs=3) as xpool,
            tc.tile_pool(name="opool", bufs=3) as opool,
            tc.tile_pool(name="ppool", bufs=8, space="PSUM") as ppool,
        ):
            # ---- k-side: Layout A, weight-stationary, fp8 DoubleRow ----
            # out[feat, rows]; stationary lhsT = W^T chunk, moving rhs = x^T
            # Weights split into 4 K-pair chunks so matmuls start gating on
            # 256 KB, not the whole 1 MB.
            wk_re = wk[:, :].rearrange("(kc p) o -> p kc o", p=128)
            wkt = []
            for i in range(KC // 2):
                t = wpool.tile([128, 2, E], dt_k, tag=f"wk{i}", name=f"wkt{i}")
                nc.sync.dma_start(t[:, :, :], wk_re[:, 2 * i:2 * i + 2, :])
                wkt.append(t)
            kinv = 1.0 / (W_SCALE if fp8_k else 1.0)
            xk_re = xk[:, :].rearrange("(kc p) r -> p kc r", p=128)
            yk_re = yk[:, :].rearrange("(jc p) r -> p jc r", p=128)
            c0 = 0
            for g, CW in enumerate(kwidths):
                ga = CW // NBLK
                xt = xpool.tile([128, KC, CW], dt_k, tag=f"xt{min(g, 3)}",
                                name="xkt")
                nc.sync.dma_start(xt[:, :, :], xk_re[:, :, c0:c0 + CW])
                for jc in range(KC):
                    pss = [ppool.tile([128, NBLK], mybir.dt.float32,
                                      tag="ps", name=f"psk{i}")
                           for i in range(ga)]
                    if fp8_k:
                        for kp in range(KC // 2):
                            lhs = wkt[kp][:, :, jc * 128:(jc + 1) * 128]
                            for i in range(ga):
                                nc.tensor.matmul(
                                    pss[i][:, :], lhs,
                                    xt[:, 2 * kp:2 * kp + 2,
                                       i * NBLK:(i + 1) * NBLK],
                                    start=(kp == 0), stop=(kp == KC // 2 - 1),
                                    perf_mode=mybir.MatmulPerfMode.DoubleRow)
                    else:
                        for kc in range(KC):
                            lhs = wkt[kc // 2][:, kc % 2,
                                               jc * 128:(jc + 1) * 128]
                            for i in range(ga):
                                nc.tensor.matmul(
                                    pss[i][:, :], lhs,
                                    xt[:, kc, i * NBLK:(i + 1) * NBLK],
                                    start=(kc == 0), stop=(kc == KC - 1))
                    ob = opool.tile([128, CW], mybir.dt.bfloat16,
                                    tag="ok", name="okt")
                    for i in range(ga):
                        nc.vector.tensor_scalar(
                            ob[:, i * NBLK:(i + 1) * NBLK],
                            pss[i][:, :], kinv, None, OP.mult)
                    nc.sync.dma_start(yk_re[:, jc, c0:c0 + CW], ob[:, :])
                c0 += CW

            # ---- v2-side: Layout B, x-stationary, bf16 ----
            wvt = wpool.tile([128, KC, E], mybir.dt.bfloat16, tag="wv",
                             name="wvt")
            nc.sync.dma_start(
                wvt[:, :, :], wv[:, :].rearrange("(kc p) o -> p kc o", p=128))
            xv_re = xv[:, :].rearrange("(kc p) r -> p kc r", p=128)
            yv_re = yv[:, :].rearrange("(rb p) e -> p rb e", p=128)
            for g in range(NGB):
                xt = xpool.tile([128, KC, RG * 128], mybir.dt.bfloat16,
                                tag="xv", name="xvt")
                nc.sync.dma_start(
                    xt[:, :, :],
                    xv_re[:, :, g * RG * 128:(g + 1) * RG * 128])
                for rb in range(RG):
                    ob = opool.tile([128, E], mybir.dt.bfloat16, tag="ov",
                                    name="ovt")
                    for half in range(2):
                        ps = ppool.tile([128, 512], mybir.dt.float32,
                                        tag="ps", name="psv")
                        for kc in range(KC):
                            nc.tensor.matmul(
                                ps[:, :],
                                xt[:, kc, rb * 128:(rb + 1) * 128],
                                wvt[:, kc, half * 512:(half + 1) * 512],
                                start=(kc == 0), stop=(kc == KC - 1))
                        nc.vector.tensor_copy(
                            ob[:, half * 512:(half + 1) * 512], ps[:, :])
                    nc.sync.dma_start(yv_re[:, g * RG + rb, :], ob[:, :])
    nc.finalize()
    return nc


def _celu_gn_rows(y, b_, g, s, out=None):
    # y: [N, E] fp32 pre-activation rows; CELU + GroupNorm(H groups).
    if np.any(b_):
        y = y + b_
    neg = np.minimum(y, 0.0)
    neg /= ALPHA
    np.expm1(neg, out=neg)
    neg *= ALPHA
    pos = np.maximum(y, 0.0, out=y if out is y else None)
    y = np.minimum(neg, 0.0, out=neg)
    y += pos
    n = y.shape[0]
    yg = y.reshape(n, H, D)
    mu = yg.mean(-1, keepdims=True)
    var = yg.var(-1, keepdims=True)
    yg -= mu
    yg /= np.sqrt(var + EPS)
    y = yg.reshape(n, E)
    if not (np.all(g == 1.0) and np.all(s == 0.0)):
        y *= g
        y += s
    if out is not None and out is not y:
        np.copyto(out, y)
    return y


def _proj_host(x, W, b, g, s):
    return _celu_gn_rows(x @ W.T.astype(x.dtype), b, g, s)


def kernel(query, key, mask, value1, value2,
           Wq, bq, gq, sq, Wk, bk, gk, sk,
           Wv1, bv1, gv1, sv1, Wv2, bv2, gv2, sv2,
           mem, Wb, bb, Wl, bl, Wl2, bl2):
    import time
    t0 = time.time()
    query = np.asarray(query, np.float32)
    key = np.asarray(key, np.float32)
    value2 = np.asarray(value2, np.float32)

    ckey = ("nc", FP8_K)
    if ckey not in _CACHE:
        _CACHE[ckey] = _build_nc(FP8_K)
    nc = _CACHE[ckey]
    t0 = _tick("build_nc", t0)

    wk_t = np.ascontiguousarray(np.asarray(Wk, np.float32).T)
    wk_t = (wk_t * W_SCALE).astype(F8) if FP8_K else wk_t.astype(BF16)
    wv_t = np.ascontiguousarray(
        np.asarray(Wv2, np.float32).T).astype(BF16)
    in_maps = []
    for c in range(NC):
        ks = key[c * BPC:(c + 1) * BPC].reshape(R, E)
        vs = value2[c * BPC:(c + 1) * BPC].reshape(R, E)
        in_maps.append({
            "xk": np.ascontiguousarray(ks.T).astype(F8 if FP8_K else BF16),
            "xv": np.ascontiguousarray(vs.T).astype(BF16),
            "wk": wk_t,
            "wv": wv_t,
        })
    t0 = _tick("in_maps prep", t0)

    res = run_bass_kernel_spmd(nc, in_maps, core_ids=list(range(NC)),
                               trace=TRACE, tmpdir=TRACE_DIR)
    _CACHE["last_res"] = res
    results = res.results
    t0 = _tick("device run", t0)

    # fused gather + CELU + GroupNorm, per-core chunks; layout [B,M,H,D]
    k = np.empty((B, M, E), np.float32)
    v2 = np.empty((B, M, E), np.float32)
    for c, r in enumerate(results):
        yk_c = np.asarray(r["yk"])                             # [E, R] bf16
        kc = k[c * BPC:(c + 1) * BPC].reshape(R, E)
        np.copyto(kc, yk_c.T)
        _celu_gn_rows(kc, bk, gk, sk, out=kc)
        yv_c = np.asarray(r["yv"])                             # [R, E] bf16
        vc = v2[c * BPC:(c + 1) * BPC].reshape(R, E)
        np.copyto(vc, yv_c)
        _celu_gn_rows(vc, bv2, gv2, sv2, out=vc)
    k = k.reshape(B, M, H, D)
    v2 = v2.reshape(B, M, H, D)
    t0 = _tick("gather+celu_gn", t0)

    q = _proj_host(query, Wq, bq, gq, sq).reshape(B, H, D)
    v1 = _proj_host(value1, Wv1, bv1, gv1, sv1).reshape(B, H, D)

    mem_hd = np.broadcast_to(mem, (B, MEM, E)).reshape(B, MEM, H, D)
    sqD = np.float32(np.sqrt(np.float32(D)))
    sqM = np.float32(np.sqrt(np.float32(MEM)))
    k = np.concatenate([k, sqD * mem_hd], axis=1)              # [B,Mt,H,D]
    mask_full = np.concatenate([mask, mask[:, :MEM]], axis=-1).astype(np.float32)
    Mt = M + MEM

    attn_map = q[:, None, :, :] * k                            # [B,Mt,H,D]
    h = attn_map.reshape(-1, D) @ Wb.T + bb                    # [BMtH, MID]
    np.maximum(h, 0.0, out=h)
    h = h.reshape(B, Mt, H, MID)
    t0 = _tick("attn_map+h", t0)

    mext = mask_full[:, :, None, None]
    pool = (h * mext).sum(axis=1) / mext.sum(axis=1)           # [B,H,MID]
    alpha_sp = h.reshape(-1, MID) @ Wl[0] + bl[0]
    alpha_sp = alpha_sp.reshape(B, Mt, H)
    alpha_sp = np.where(mask_full[:, :, None] == 0, np.float32(-1e9), alpha_sp)
    alpha_sp = alpha_sp - alpha_sp.max(1, keepdims=True)
    np.exp(alpha_sp, out=alpha_sp)
    alpha_sp /= alpha_sp.sum(1, keepdims=True)
    alpha_ch = 1.0 / (1.0 + np.exp(-(pool @ Wl2.T + bl2)))     # [B,H,D]
    v2p = np.einsum("bmh,bmhd->bhd", alpha_sp[:, :M], v2, optimize=True)
    v2p += np.einsum("bmh,bmhd->bhd", alpha_sp[:, M:], sqM * mem_hd,
                     optimize=True)
    attn = v1 * v2p * alpha_ch
    _tick("rest of epilogue", t0)
    return attn.reshape(B, E).astype(np.float32)



# revision 3
# speedup vs baseline: 1.0650x; 1.0650x over previous
"""Trainium2 kernel for nn_LowRank (sparse_attention).

Strategy: data-parallel over batch B=128 across 8 NeuronCores (16 rows each).
The two dominant Linear projections (key/value2: 2 x 137 GMAC, ~95% of FLOPs)
run on-device:
  - k-side in fp8-e4m3 with DoubleRow perf mode (2 MACs/cell/cycle).  A
    numerical simulation shows k-side fp8 is safe (2.2e-3 end-to-end vs
    2.1e-3 for bf16): k-path errors are damped by the near-uniform softmax
    and the sigmoid channel gate.  Weights are pre-scaled by 64 to stay out
    of fp8 subnormals; the PSUM->SBUF copy divides it back out.
  - v2-side in bf16 (fp8 here fails the tolerance: v2 feeds the output
    directly; measured 3.8e-2 vs the 2e-2 gate, and even one fp8 operand
    alone is 2.3-2.4e-2).
The cheap epilogue (CELU, GroupNorm, SCAttention) runs on host in fp32.

All device I/O is pre-tiled on the host so every DMA is a fully
contiguous [128, n] transfer (one descriptor chain per partition, no
strided access patterns): this keeps the sync engine off the tensor
engine's critical path.  The tensor engine streams 3072 matmuls at the
216 ns/512-col hardware rate (~663 us); head DMA (~12 us) and tail
drain (~10 us) are minimized by a small first weight chunk and
per-row-block stores for the final output group.
"""

import sys

for p in ("/opt/trn_rl_repo",):
    if p not in sys.path:
        sys.path.insert(0, p)

import numpy as np
import ml_dtypes

import concourse.bass as bass
import concourse.bacc as bacc
import concourse.mybir as mybir
from concourse import tile
from concourse.bass_utils import run_bass_kernel_spmd

BF16 = ml_dtypes.bfloat16
F8 = ml_dtypes.float8_e4m3

B, M, E, H, MEM = 128, 1024, 1024, 8, 40
D = E // H
MID = 64
ALPHA = 1.3
EPS = 1e-5
NC = 8
BPC = B // NC          # 16 batch rows per core
R = BPC * M            # 16384 rows of x per core
KC = E // 128          # 8 contraction chunks
W_SCALE = 64.0         # fp8 weight pre-scale (keeps W out of subnormals)
FP8_K = True           # k-side projection in fp8 DoubleRow

# k-side (Layout A: features on partitions, weight-stationary, fp8 DR)
NBLK = 512             # columns (rows of x) per psum tile
CWK = 1024             # columns per k-side DMA group
NGA = R // CWK         # 16 column groups
GA = CWK // NBLK       # psum tiles in flight per (group, jc)

# v2-side (Layout B: rows on partitions, bf16)
RG = 8                 # row-blocks of 128 per DMA group
CWV = RG * 128         # 1024
NGB = R // CWV         # 16 groups

_CACHE = {}
TRACE = False          # test.py sets True to capture an NTFF profile
TRACE_DIR = None
VERBOSE = False


def _tick(msg, t0):
    import time
    t = time.time()
    if VERBOSE:
        print(f"[kernel] {msg}: {t - t0:.2f}s", flush=True)
    return t


def _build_nc(fp8_k=FP8_K):
    OP = mybir.AluOpType
    nc = bacc.Bacc(trn_type="TRN2")
    dt_k = mybir.dt.float8e4 if fp8_k else mybir.dt.bfloat16
    # Pre-tiled layouts (host side does the shuffles):
    #   xk[p, g*KC+kc, c] = key^T[kc*128+p, g*CWK+c]
    #   wk[p, kc, o]      = Wk^T[kc*128+p, o] * W_SCALE
    #   yk[p, g*KC+jc, c] = (Wk @ key^T)[jc*128+p, g*CWK+c]
    #   xv[p, g*KC+kc, c] = value2^T[kc*128+p, g*CWV+c]
    #   wv[p, kc, o]      = Wv2^T[kc*128+p, o]
    #   yv[p, rb, e]      = (value2 @ Wv2^T)[rb*128+p, e]
    xk = nc.dram_tensor("xk", (128, NGA * KC, CWK), dt_k,
                        kind="ExternalInput")
    wk = nc.dram_tensor("wk", (128, KC, E), dt_k, kind="ExternalInput")
    yk = nc.dram_tensor("yk", (128, NGA * KC, CWK), mybir.dt.bfloat16,
                        kind="ExternalOutput")
    xv = nc.dram_tensor("xv", (128, NGB * KC, CWV), mybir.dt.bfloat16,
                        kind="ExternalInput")
    wv = nc.dram_tensor("wv", (128, KC, E), mybir.dt.bfloat16,
                        kind="ExternalInput")
    yv = nc.dram_tensor("yv", (128, R // 128, E), mybir.dt.bfloat16,
                        kind="ExternalOutput")

    with tile.TileContext(nc) as tc:
        with (
            tc.tile_pool(name="wpool", bufs=1) as wpool,
            tc.tile_pool(name="xpool", bufs=3) as xpool,
            tc.tile_pool(name="opool", bufs=2) as opool,
            tc.tile_pool(name="ppool", bufs=8, space="PSUM") as ppool,
        ):
            # ---- k-side: Layout A, weight-stationary, fp8 DoubleRow ----
            # out[feat, rows]; stationary lhsT = W^T chunk, moving rhs = x^T.
            # Weights in 4 K-pair chunks so the first matmul gates on 256 KB.
            wkt = []
            for i in range(KC // 2):
                t = wpool.tile([128, 2, E], dt_k, tag=f"wk{i}", name=f"wkt{i}")
                nc.sync.dma_start(t[:, :, :], wk[:, 2 * i:2 * i + 2, :])
                wkt.append(t)
            kinv = 1.0 / (W_SCALE if fp8_k else 1.0)
            for g in range(NGA):
                xt = xpool.tile([128, KC, CWK], dt_k, tag="xt", name="xkt")
                nc.sync.dma_start(xt[:, :, :],
                                  xk[:, g * KC:(g + 1) * KC, :])
                ob = opool.tile([128, KC, CWK], mybir.dt.bfloat16, tag="ot",
                                name="okt")
                for jc in range(KC):
                    pss = [ppool.tile([128, NBLK], mybir.dt.float32,
                                      tag="ps", name=f"psk{i}")
                           for i in range(GA)]
                    if fp8_k:
                        for kp in range(KC // 2):
                            lhs = wkt[kp][:, :, jc * 128:(jc + 1) * 128]
                            for i in range(GA):
                                nc.tensor.matmul(
                                    pss[i][:, :], lhs,
                                    xt[:, 2 * kp:2 * kp + 2,
                                       i * NBLK:(i + 1) * NBLK],
                                    start=(kp == 0), stop=(kp == KC // 2 - 1),
                                    perf_mode=mybir.MatmulPerfMode.DoubleRow)
                    else:
                        for kc in range(KC):
                            lhs = wkt[kc // 2][:, kc % 2,
                                               jc * 128:(jc + 1) * 128]
                            for i in range(GA):
                                nc.tensor.matmul(
                                    pss[i][:, :], lhs,
                                    xt[:, kc, i * NBLK:(i + 1) * NBLK],
                                    start=(kc == 0), stop=(kc == KC - 1))
                    for i in range(GA):
                        nc.vector.tensor_scalar(
                            ob[:, jc, i * NBLK:(i + 1) * NBLK],
                            pss[i][:, :], kinv, None, OP.mult)
                nc.sync.dma_start(yk[:, g * KC:(g + 1) * KC, :], ob[:, :, :])

            # ---- v2-side: Layout B, x-stationary, bf16 ----
            wvt = wpool.tile([128, KC, E], mybir.dt.bfloat16, tag="wv",
                             name="wvt")
            nc.sync.dma_start(wvt[:, :, :], wv[:, :, :])
            for g in range(NGB):
                last = g == NGB - 1
                xt = xpool.tile([128, KC, CWV], mybir.dt.bfloat16,
                                tag="xt", name="xvt")
                nc.sync.dma_start(xt[:, :, :],
                                  xv[:, g * KC:(g + 1) * KC, :])
                ob = None
                if not last:
                    ob = opool.tile([128, RG, E], mybir.dt.bfloat16,
                                    tag="ot", name="ovt")
                for rb in range(RG):
                    if last:
                        # fine-grained stores so the final drain is ~1 store
                        obr = opool.tile([128, E], mybir.dt.bfloat16,
                                         tag="ovr", name="ovr")
                    for half in range(2):
                        ps = ppool.tile([128, 512], mybir.dt.float32,
                                        tag="ps", name="psv")
                        for kc in range(KC):
                            nc.tensor.matmul(
                                ps[:, :],
                                xt[:, kc, rb * 128:(rb + 1) * 128],
                                wvt[:, kc, half * 512:(half + 1) * 512],
                                start=(kc == 0), stop=(kc == KC - 1))
                        dst = (obr[:, half * 512:(half + 1) * 512] if last
                               else ob[:, rb, half * 512:(half + 1) * 512])
                        nc.vector.tensor_copy(dst, ps[:, :])
                    if last:
                        nc.sync.dma_start(yv[:, g * RG + rb, :], obr[:, :])
                if not last:
                    nc.sync.dma_start(yv[:, g * RG:(g + 1) * RG, :],
                                      ob[:, :, :])
    nc.finalize()
    return nc


def _celu_gn_rows(y, b_, g, s, out=None):
    # y: [N, E] fp32 pre-activation rows; CELU + GroupNorm(H groups).
    if np.any(b_):
        y = y + b_
    neg = np.minimum(y, 0.0)
    neg /= ALPHA
    np.expm1(neg, out=neg)
    neg *= ALPHA
    pos = np.maximum(y, 0.0, out=y if out is y else None)
    y = np.minimum(neg, 0.0, out=neg)
    y += pos
    n = y.shape[0]
    yg = y.reshape(n, H, D)
    mu = yg.mean(-1, keepdims=True)
    var = yg.var(-1, keepdims=True)
    yg -= mu
    yg /= np.sqrt(var + EPS)
    y = yg.reshape(n, E)
    if not (np.all(g == 1.0) and np.all(s == 0.0)):
        y *= g
        y += s
    if out is not None and out is not y:
        np.copyto(out, y)
    return y


def _proj_host(x, W, b, g, s):
    return _celu_gn_rows(x @ W.T.astype(x.dtype), b, g, s)


def _tile_x(xrows, cw, dt):
    # xrows [R, E] -> [128, (R//cw)*KC, cw] with
    # out[p, g*KC+kc, c] = xrows[g*cw+c, kc*128+p]
    ng = R // cw
    return np.ascontiguousarray(
        xrows.reshape(ng, cw, KC, 128).transpose(3, 0, 2, 1)
    ).reshape(128, ng * KC, cw).astype(dt)


def _tile_w(wt, dt):
    # wt [E_in, E_out] -> [128, KC, E] with out[p, kc, o] = wt[kc*128+p, o]
    return np.ascontiguousarray(
        wt.reshape(KC, 128, E).transpose(1, 0, 2)).astype(dt)


def kernel(query, key, mask, value1, value2,
           Wq, bq, gq, sq, Wk, bk, gk, sk,
           Wv1, bv1, gv1, sv1, Wv2, bv2, gv2, sv2,
           mem, Wb, bb, Wl, bl, Wl2, bl2):
    import time
    t0 = time.time()
    query = np.asarray(query, np.float32)
    key = np.asarray(key, np.float32)
    value2 = np.asarray(value2, np.float32)

    ckey = ("nc", FP8_K)
    if ckey not in _CACHE:
        _CACHE[ckey] = _build_nc(FP8_K)
    nc = _CACHE[ckey]
    t0 = _tick("build_nc", t0)

    wk_t = np.ascontiguousarray(np.asarray(Wk, np.float32).T)
    wk_t = _tile_w(wk_t * W_SCALE if FP8_K else wk_t, F8 if FP8_K else BF16)
    wv_t = _tile_w(np.ascontiguousarray(np.asarray(Wv2, np.float32).T), BF16)
    in_maps = []
    for c in range(NC):
        ks = key[c * BPC:(c + 1) * BPC].reshape(R, E)
        vs = value2[c * BPC:(c + 1) * BPC].reshape(R, E)
        in_maps.append({
            "xk": _tile_x(ks, CWK, F8 if FP8_K else BF16),
            "xv": _tile_x(vs, CWV, BF16),
            "wk": wk_t,
            "wv": wv_t,
        })
    t0 = _tick("in_maps prep", t0)

    res = run_bass_kernel_spmd(nc, in_maps, core_ids=list(range(NC)),
                               trace=TRACE, tmpdir=TRACE_DIR)
    _CACHE["last_res"] = res
    results = res.results
    t0 = _tick("device run", t0)

    # fused gather + CELU + GroupNorm, per-core chunks; layout [B,M,H,D]
    k = np.empty((B, M, E), np.float32)
    v2 = np.empty((B, M, E), np.float32)
    for c, r in enumerate(results):
        yk_c = np.asarray(r["yk"])          # [128, NGA*KC, CWK] bf16
        kc = k[c * BPC:(c + 1) * BPC].reshape(R, E)
        np.copyto(kc, yk_c.reshape(128, NGA, KC, CWK)
                  .transpose(1, 3, 2, 0).reshape(R, E))
        _celu_gn_rows(kc, bk, gk, sk, out=kc)
        yv_c = np.asarray(r["yv"])          # [128, R//128, E] bf16
        vc = v2[c * BPC:(c + 1) * BPC].reshape(R, E)
        np.copyto(vc, yv_c.transpose(1, 0, 2).reshape(R, E))
        _celu_gn_rows(vc, bv2, gv2, sv2, out=vc)
    k = k.reshape(B, M, H, D)
    v2 = v2.reshape(B, M, H, D)
    t0 = _tick("gather+celu_gn", t0)

    q = _proj_host(query, Wq, bq, gq, sq).reshape(B, H, D)
    v1 = _proj_host(np.asarray(value1, np.float32), Wv1, bv1, gv1,
                    sv1).reshape(B, H, D)

    mem_hd = np.broadcast_to(mem, (B, MEM, E)).reshape(B, MEM, H, D)
    sqD = np.float32(np.sqrt(np.float32(D)))
    sqM = np.float32(np.sqrt(np.float32(MEM)))
    k = np.concatenate([k, sqD * mem_hd], axis=1)              # [B,Mt,H,D]
    mask_full = np.concatenate([mask, mask[:, :MEM]], axis=-1).astype(np.float32)
    Mt = M + MEM

    attn_map = q[:, None, :, :] * k                            # [B,Mt,H,D]
    h = attn_map.reshape(-1, D) @ Wb.T + bb                    # [BMtH, MID]
    np.maximum(h, 0.0, out=h)
    h = h.reshape(B, Mt, H, MID)
    t0 = _tick("attn_map+h", t0)

    mext = mask_full[:, :, None, None]
    pool = (h * mext).sum(axis=1) / mext.sum(axis=1)           # [B,H,MID]
    alpha_sp = h.reshape(-1, MID) @ Wl[0] + bl[0]
    alpha_sp = alpha_sp.reshape(B, Mt, H)
    alpha_sp = np.where(mask_full[:, :, None] == 0, np.float32(-1e9), alpha_sp)
    alpha_sp = alpha_sp - alpha_sp.max(1, keepdims=True)
    np.exp(alpha_sp, out=alpha_sp)
    alpha_sp /= alpha_sp.sum(1, keepdims=True)
    alpha_ch = 1.0 / (1.0 + np.exp(-(pool @ Wl2.T + bl2)))     # [B,H,D]
    v2p = np.einsum("bmh,bmhd->bhd", alpha_sp[:, :M], v2, optimize=True)
    v2p += np.einsum("bmh,bmhd->bhd", alpha_sp[:, M:], sqM * mem_hd,
                     optimize=True)
    attn = v1 * v2p * alpha_ch
    _tick("rest of epilogue", t0)
    return attn.reshape(B, E).astype(np.float32)
